# revision 48
# baseline (speedup 1.0000x reference)
"""DreamAttention sparse-attention kernel for 8 Trainium2 NeuronCores.

Sharding: tensor-parallel over heads. Core c owns kv-head c and q-heads
(2c, 2c+1). Each core projects q for all tokens (its head pair), projects
k/v for the salient rows (its kv head), applies RoPE, and runs full
bidirectional GQA attention for its heads. The per-head attention
outputs (kept in o^T layout, bf16) are re-sharded token-wise with an
on-device AllToAll, after which every core computes the full o_proj for
its 512-token slice; the host concatenates the 8 row slices.

Cache-update-without-scatter: stale salient rows of the previous cache
are excluded from the softmax by an additive -60 bias inside
exp(scale*x + bias) (per-key-tile bias vectors indexed by partition =
key). The freshly projected salient keys/values enter as an extra dense
key block, host-reordered so batch-0 keys occupy the first t0 128-key
tiles and batch-1 keys the next t1 tiles (pads killed by the same -60
bias), so each batch only attends its own salient tiles.

Q/K/V projections run in fp8 e4m3 with DoubleRow perf mode (two 128-row
contraction tiles per PE pass): the host scales h by 16 and the weights
by 32 (exact powers of two) and the kernel folds 1/512 into the
bias-add stage, keeping everything well inside e4m3's normal range.

Softmax: scores for a query-chunk pair live in one [128, 1024] PSUM
tile, one Exp activation produces the paired prob tile (bf16), and the
denominators accumulate OFF the tensor engine: even key tiles are
summed on DVE, odd tiles on gpsimd (bf16 pairwise adds), merged, then
reduced across partitions with one ones-vector matmul per 512-query
chunk, inverted with reciprocal_approx_fast, broadcast, and multiplied
into the o^T PSUM accumulator on DVE.
"""

import os
import sys

for _p in ("/opt/trn_rl_repo", "/root/.axon_site/_ro/trn_rl_repo"):
    if os.path.isdir(_p) and _p not in sys.path:
        sys.path.insert(0, _p)

import numpy as np
import ml_dtypes

import concourse.bacc as bacc
import concourse.mybir as mybir
import concourse.tile as tile
from concourse import bass_isa, bass_utils

B, L = 2, 2048
T = B * L
HIDDEN = 2048
H, HKV, D = 16, 8, 128
S = 1024
ROPE_BASE = 1000000.0
HALF = D // 2
N_CORES = 8
G = H // HKV              # q heads per core (= per kv head)
DOUT = G * D              # 256 q-proj cols per core
TPC = T // N_CORES        # 512 output token rows per core
NKT = HIDDEN // 128       # 16 contraction tiles
NKP = NKT // 2            # 8 DoubleRow contraction pairs
SCALE = float(D) ** -0.5
NEG = -60.0               # kills stale/pad keys inside exp
NST = L // 128            # 16 prev key tiles per batch
IC = 512                  # query chunk
NIC = L // IC             # 4 chunks per batch
HSC = 16.0                # host scale on h (exact power of 2)
WSC = 32.0                # host scale on Wq/Wkv
ISC = 1.0 / (HSC * WSC)   # folded back after PSUM accumulation

F32 = mybir.dt.float32
F32R = mybir.dt.float32r
BF16 = mybir.dt.bfloat16
FP8 = mybir.dt.float8e4
DR = mybir.MatmulPerfMode.DoubleRow

_cache = {}


def _rope_apply(nc, out_ap, x_ap, xsw_ap, cs1_ap, cs2_ap, tmp_ap):
    """NeoX rope in [d, token] layout, same-partition form.

    out = x * [cos;cos] + swap(x) * [-sin;sin], where swap(x) (the two
    d-halves exchanged) was produced by a PE matmul with a permutation
    matrix, so every DVE operand here starts at partition 0.
    """
    mul = mybir.AluOpType.mult
    add = mybir.AluOpType.add
    nc.vector.tensor_tensor(tmp_ap, xsw_ap, cs2_ap, mul)
    nc.vector.tensor_tensor(out_ap, x_ap, cs1_ap, mul)
    nc.vector.tensor_tensor(out_ap, out_ap, tmp_ap, add)


def _build(t0, t1):
    nc = bacc.Bacc("TRN2", target_bir_lowering=False, debug=False,
                   num_devices=N_CORES)

    SJT = t0 + t1             # total salient key tiles
    SSL = SJT * 128           # salient slots
    SP = ((SSL + 511) // 512) * 512   # kv-proj column count (512-aligned)
    NTB = (NST + max(t0, t1)) # bias columns per batch
    NPE = 0                   # denominator key tiles handled on the PE

    # ---- DRAM I/O (per-core shards prepared by the host) ----
    hT = nc.dram_tensor("hT", [HIDDEN, T], FP8, kind="ExternalInput").ap()
    hsalT = nc.dram_tensor("hsalT", [HIDDEN + 1, SP], FP8, kind="ExternalInput").ap()
    wq = nc.dram_tensor("wq", [128, NKP, 2, DOUT], FP8, kind="ExternalInput").ap()
    bq = nc.dram_tensor("bq", [G, 128, 1], F32, kind="ExternalInput").ap()
    wk = nc.dram_tensor("wk", [128, NKP, 2, D], FP8, kind="ExternalInput").ap()
    bk = nc.dram_tensor("bk", [128, 1], F32, kind="ExternalInput").ap()
    wv = nc.dram_tensor("wv", [128, NKP, 2, D], FP8, kind="ExternalInput").ap()
    wvl = nc.dram_tensor("wvl", [1, D], FP8, kind="ExternalInput").ap()
    wo = nc.dram_tensor("wo", [HIDDEN, HIDDEN], F32R, kind="ExternalInput").ap()
    kpT = nc.dram_tensor("kpT", [B, D, L], BF16, kind="ExternalInput").ap()
    vpa = nc.dram_tensor("vpa", [B, L, D], BF16, kind="ExternalInput").ap()
    onec = nc.dram_tensor("onec", [128, 1], BF16, kind="ExternalInput").ap()
    onesr = nc.dram_tensor("onesr", [1, 128], F32R, kind="ExternalInput").ap()
    csq1 = nc.dram_tensor("csq1", [D, T], BF16, kind="ExternalInput").ap()
    csq2 = nc.dram_tensor("csq2", [D, T], BF16, kind="ExternalInput").ap()
    css1 = nc.dram_tensor("css1", [D, SP], BF16, kind="ExternalInput").ap()
    css2 = nc.dram_tensor("css2", [D, SP], BF16, kind="ExternalInput").ap()
    swm = nc.dram_tensor("swm", [D, D], BF16, kind="ExternalInput").ap()
    idm = nc.dram_tensor("idm", [D, D], BF16, kind="ExternalInput").ap()
    abias = nc.dram_tensor("abias", [B, 128, NTB], F32, kind="ExternalInput").ap()
    out = nc.dram_tensor("out", [TPC, HIDDEN], F32, kind="ExternalOutput").ap()

    Exp = mybir.ActivationFunctionType.Exp
    Copy = mybir.ActivationFunctionType.Copy
    mul = mybir.AluOpType.mult
    add = mybir.AluOpType.add

    with tile.TileContext(nc) as tc:
        with (
            tc.tile_pool(name="consts", bufs=1) as consts,
            tc.tile_pool(name="dram", bufs=1, space="DRAM") as dram,
        ):
            ident = consts.tile([128, 128], BF16)
            swm_t = consts.tile([D, D], BF16)
            onec_t = consts.tile([128, 1], BF16)
            onesr_t = consts.tile([1, 128], F32R)
            css1_t = consts.tile([D, SP], BF16)
            css2_t = consts.tile([D, SP], BF16)
            abias_t = [consts.tile([128, NTB], F32, name=f"abias{b}")
                       for b in range(B)]
            bq_t = [consts.tile([128, 1], F32, name=f"bqt{g}") for g in range(G)]
            bk_t = consts.tile([128, 1], F32)

            # o^T stacked layout: block j (token chunk j) = this core's
            # head g's UNNORMALIZED o^T columns (bf16) for 256 tokens of
            # batch b, plus two denominator rows per block (hi/lo bf16
            # split, so the reconstructed f32 denominator is ~exact);
            # normalization commutes with the A2A and is applied on the
            # receiving side
            a2a_in = [dram.tile([N_CORES * (D + 2), TPC // B], BF16,
                                name=f"a2a_in{i}") for i in range(G * B)]
            a2a_out = [dram.tile([N_CORES * (D + 2), TPC // B], BF16,
                                 name=f"a2a_out{i}") for i in range(G * B)]

            wost_cm = tc.tile_pool(name="wost", bufs=32)
            wost = wost_cm.__enter__()
            with (
                tc.tile_pool(name="wqp", bufs=1) as wqp,
                tc.tile_pool(name="wkvp", bufs=1) as wkvp,
                tc.tile_pool(name="kvres", bufs=1) as kvres,
                tc.tile_pool(name="qres", bufs=1) as qres,
            ):
                wk_s = wkvp.tile([128, NKP, 2, D], FP8)
                wv_s = wkvp.tile([128, NKP, 2, D], FP8)
                wv_last = wkvp.tile([1, D], FP8)
                nc.sync.dma_start(wk_s[:], wk[:])
                nc.scalar.dma_start(wv_s[:], wv[:])
                nc.scalar.dma_start(wv_last[:], wvl[:])
                wq_s = wqp.tile([128, NKP, 2, DOUT], FP8)
                nc.gpsimd.dma_start(wq_s[:], wq[:])
                # small consts on the gpsimd queue so the sync/scalar queues
                # stream hsalT immediately
                nc.gpsimd.dma_start(swm_t[:], swm[:])
                nc.gpsimd.dma_start(ident[:], idm[:])
                nc.gpsimd.dma_start(bk_t[:], bk[:])
                nc.gpsimd.dma_start(onec_t[:], onec[:])
                nc.gpsimd.dma_start(onesr_t[:], onesr[:])
                for g in range(G):
                    nc.gpsimd.dma_start(bq_t[g][:], bq[g])
                for b in range(B):
                    nc.gpsimd.dma_start(abias_t[b][:], abias[b])
                nc.gpsimd.dma_start(css1_t[:], css1[:])
                nc.gpsimd.dma_start(css2_t[:], css2[:])

                # Residents: prev-cache K^T and V rows per batch, new K^T
                # (roped) and new V rows.
                kpT_t = [kvres.tile([D, L], BF16, name=f"kpTt{b}")
                         for b in range(B)]
                vpa_t = [kvres.tile([128, NST * D], BF16, name=f"vpat{b}")
                         for b in range(B)]
                knT_t = kvres.tile([D, SSL], BF16)
                vnew_t = [kvres.tile([128, D], BF16, name=f"vnewt{j}")
                          for j in range(SJT)]

                # ---- S2: kv projection for salient rows (fp8 DoubleRow) ----
                with (
                    tc.tile_pool(name="hsal", bufs=4) as hsalp,
                    tc.tile_pool(name="s2sb", bufs=1) as s2sb,
                    tc.tile_pool(name="kvps", bufs=1, space="PSUM") as kvps,
                ):
                    kn_ps = kvps.tile([D, SP], F32)
                    vt_ps = kvps.tile([D, SP], F32)
                    for p in range(NKP):
                        hs = hsalp.tile([128, 2, SP], FP8, tag="hs")
                        heng = nc.sync if p % 2 == 0 else nc.scalar
                        heng.dma_start(
                            hs[:], hsalT[p * 256:(p + 1) * 256, :]
                            .rearrange("(two p) n -> p two n", p=128))
                        for n in range(SP // 512):
                            sl = slice(n * 512, (n + 1) * 512)
                            nc.tensor.matmul(kn_ps[:, sl], wk_s[:, p],
                                             hs[:, :, sl], perf_mode=DR,
                                             start=(p == 0), stop=(p == NKP - 1))
                            nc.tensor.matmul(vt_ps[:, sl], wv_s[:, p],
                                             hs[:, :, sl], perf_mode=DR,
                                             start=(p == 0), stop=False)
                    hlast = hsalp.tile([1, SP], FP8, tag="hl")
                    nc.sync.dma_start(hlast[:], hsalT[HIDDEN:HIDDEN + 1, :])
                    for n in range(SP // 512):
                        sl = slice(n * 512, (n + 1) * 512)
                        nc.tensor.matmul(vt_ps[:, sl], wv_last[:], hlast[:, sl],
                                         start=False, stop=True)
                    # K: scale-fold + bias then rope into knT_t
                    knraw = s2sb.tile([D, SP], BF16)
                    nc.vector.tensor_scalar(knraw[:], kn_ps[:], ISC,
                                            bk_t[:, 0:1], mul, add)
                    ktmp = s2sb.tile([D, SP], BF16)
                    with tc.tile_pool(name="kswp", bufs=2, space="PSUM") as kswp:
                        for n in range(SP // 512):
                            sl = slice(n * 512, (n + 1) * 512)
                            ksw_ps = kswp.tile([D, 512], F32, tag="ksw")
                            nc.tensor.matmul(ksw_ps[:], swm_t[:],
                                             knraw[:, sl], start=True, stop=True)
                            if n * 512 < SSL:
                                osl = slice(n * 512, min((n + 1) * 512, SSL))
                                w = osl.stop - osl.start
                                _rope_apply(nc, knT_t[:, osl], knraw[:, osl],
                                            ksw_ps[:, 0:w], css1_t[:, osl],
                                            css2_t[:, osl], ktmp[:, osl])
                    # V: scale-fold out of PSUM, transpose to row tiles
                    vtS = s2sb.tile([D, SP], BF16)
                    nc.scalar.activation(vtS[:], vt_ps[:], Copy, scale=ISC)
                    with tc.tile_pool(name="vtrp", bufs=2, space="PSUM") as vtrp:
                        for jt in range(SJT):
                            tp = vtrp.tile([128, 128], BF16, tag="tp")
                            nc.tensor.transpose(
                                tp[:], vtS[:, jt * 128:(jt + 1) * 128], ident[:])
                            nc.vector.tensor_copy(vnew_t[jt][:], tp[:])

                # residents for S4 queued after the S2 streams
                for b in range(B):
                    nc.gpsimd.dma_start(kpT_t[b][:], kpT[b])
                    nc.gpsimd.dma_start(
                        vpa_t[b][:].rearrange("p (s d) -> p s d", d=D),
                        vpa[b].rearrange("(s p) d -> p s d", p=128))

                # ---- S3: q projection (fp8 DoubleRow) + rope ----
                hstr_cm = tc.tile_pool(name="hstr", bufs=8)
                hstr = hstr_cm.__enter__()
                qT_t = [qres.tile([D, T], BF16, name=f"qTt{g}") for g in range(G)]
                with (
                    tc.tile_pool(name="csqp", bufs=1) as csqp,
                    tc.tile_pool(name="qraw", bufs=4) as qrawp,
                    tc.tile_pool(name="qps", bufs=4, space="PSUM") as qps,
                    tc.tile_pool(name="qswps", bufs=2, space="PSUM") as qswps,
                ):
                    csq1_t = csqp.tile([D, T], BF16)
                    csq2_t = csqp.tile([D, T], BF16)
                    nc.gpsimd.dma_start(csq1_t[:], csq1[:])
                    nc.gpsimd.dma_start(csq2_t[:], csq2[:])

                    def rope_flush(n, q_ps):
                        sl = slice(n * 512, (n + 1) * 512)
                        for g in range(G):
                            qraw = qrawp.tile([128, 512], BF16, tag="qr")
                            nc.vector.tensor_scalar(qraw[:], q_ps[g][:], ISC,
                                                    bq_t[g][:, 0:1], mul, add)
                            qsw_ps = qswps.tile([128, 512], F32, tag="qsw")
                            nc.tensor.matmul(qsw_ps[:], swm_t[:], qraw[:],
                                             start=True, stop=True)
                            qtmp = qrawp.tile([128, 512], BF16, tag="qtmp")
                            _rope_apply(nc, qT_t[g][:, sl], qraw[:], qsw_ps[:],
                                        csq1_t[:, sl], csq2_t[:, sl], qtmp[:])

                    qpend = None
                    for n in range(T // 512):
                        sl = slice(n * 512, (n + 1) * 512)
                        q_ps = [qps.tile([128, 512], F32, tag="qp",
                                         name=f"qps{g}") for g in range(G)]
                        for p in range(NKP):
                            ht = hstr.tile([128, 2, 512], FP8, tag="ht")
                            eng = nc.sync if p % 2 == 0 else nc.scalar
                            eng.dma_start(
                                ht[:], hT[p * 256:(p + 1) * 256, sl]
                                .rearrange("(two p) n -> p two n", p=128))
                            for g in range(G):
                                nc.tensor.matmul(
                                    q_ps[g][:],
                                    wq_s[:, p, :, g * 128:(g + 1) * 128],
                                    ht[:], perf_mode=DR,
                                    start=(p == 0), stop=(p == NKP - 1))
                        if qpend is not None:
                            rope_flush(*qpend)
                        qpend = (n, q_ps)
                    rope_flush(*qpend)

                hstr_cm.__exit__(None, None, None)

                # ---- S4: attention, o^T accumulated V-stationary ----
                # o_proj weight tiles stream on the gpsimd queue, a few per
                # attention chunk, so no queue sees a burst
                wo_t = {}
                wo_iter = [(hc, dt) for hc in range(HIDDEN // 512)
                           for dt in range(NKT)]

                def wo_prefetch(k):
                    for hc, dt in wo_iter[8 * k:8 * k + 8]:
                        w = wost.tile([128, 512], F32R, tag="wot")
                        nc.sync.dma_start(
                            w[:], wo[dt * 128:(dt + 1) * 128,
                                     hc * 512:(hc + 1) * 512])
                        wo_t[(hc, dt)] = w
                NGP = 4   # leading key tiles summed on gpsimd; rest on DVE
                with (
                    tc.tile_pool(name="ptp", bufs=5) as ptp,
                    tc.tile_pool(name="accp", bufs=4) as accp,
                    tc.tile_pool(name="oscp", bufs=3) as oscp,
                    tc.tile_pool(name="dsbp", bufs=4) as dsbp,
                    tc.tile_pool(name="scps", bufs=2, space="PSUM") as scps,
                    tc.tile_pool(name="opps", bufs=3, space="PSUM") as opps,
                    tc.tile_pool(name="dnps", bufs=1, space="PSUM") as dnps,
                ):
                    def den_flush(b, g, ics, acc, last_of_gb):
                        # partition-reduce the merged prob sums (two 512-row
                        # ones matmuls) and ship the denominator rows; only
                        # gates the A2A trigger, never the o^T accumulators
                        dn_ps = dnps.tile([64, IC], F32, tag="dn")
                        for x in range(2):
                            nc.tensor.matmul(
                                dn_ps[32 * x:32 * x + 1, :], onec_t[:],
                                acc[0][:, x * IC:(x + 1) * IC],
                                start=True, stop=True)
                        buf = a2a_in[g * B + b]
                        hwc = TPC // B
                        for x in range(2):
                            dhi = dsbp.tile([1, IC], BF16, tag="dhi")
                            nc.vector.tensor_copy(
                                dhi[:], dn_ps[32 * x:32 * x + 1, :])
                            dlo = dsbp.tile([1, IC], BF16, tag="dlo")
                            nc.vector.tensor_tensor(
                                dlo[:], dn_ps[32 * x:32 * x + 1, :], dhi[:],
                                mybir.AluOpType.subtract)
                            for hh in range(2):
                                r0 = (2 * ics[x] + hh) * (D + 2) + D
                                nc.sync.dma_start(
                                    buf[r0:r0 + 1, :],
                                    dhi[:, hh * hwc:(hh + 1) * hwc])
                                nc.sync.dma_start(
                                    buf[r0 + 1:r0 + 2, :],
                                    dlo[:, hh * hwc:(hh + 1) * hwc])
                        if last_of_gb:
                            # token re-shard for (g, b); runs on the
                            # TOPSP/SDMA path while the PE keeps computing.
                            nc.gpsimd.collective_compute(
                                "AllToAll", mybir.AluOpType.bypass,
                                ins=[a2a_in[g * B + b].opt()],
                                outs=[a2a_out[g * B + b].opt()],
                                replica_groups=[list(range(N_CORES))],
                            )

                    pend = None
                    chunk_i = 0
                    for b in range(B):
                        joff = 0 if b == 0 else t0
                        ntot = NST + (t0 if b == 0 else t1)
                        for g in range(G):
                            for icp in range(NIC // 2):
                                ics = (2 * icp, 2 * icp + 1)
                                qsls = [slice(b * L + ic * IC,
                                              b * L + (ic + 1) * IC)
                                        for ic in ics]
                                op_ps = [opps.tile([128, IC], F32, tag="op",
                                                   name=f"op{x}")
                                         for x in range(2)]
                                acc = [accp.tile([128, 2 * IC], BF16, tag="acc",
                                                 name=f"acc{e}")
                                       for e in range(2)]
                                nc.gpsimd.memset(acc[1][:], 0.0)
                                for st in range(ntot):
                                    if st < NST:
                                        ktile = kpT_t[b][:, st * 128:(st + 1) * 128]
                                        vtile = vpa_t[b][:, st * D:(st + 1) * D]
                                    else:
                                        jt = joff + (st - NST)
                                        ktile = knT_t[:, jt * 128:(jt + 1) * 128]
                                        vtile = vnew_t[jt][:]
                                    bcol = abias_t[b][:, st:st + 1]
                                    sc2 = scps.tile([128, 2 * IC], F32, tag="sc")
                                    for x in range(2):
                                        nc.tensor.matmul(
                                            sc2[:, x * IC:(x + 1) * IC], ktile,
                                            qT_t[g][:, qsls[x]],
                                            start=True, stop=True)
                                    pt2 = ptp.tile([128, 2 * IC], BF16, tag="pt")
                                    nc.scalar.activation(pt2[:], sc2[:], Exp,
                                                         scale=SCALE, bias=bcol)
                                    for x in range(2):
                                        nc.tensor.matmul(
                                            op_ps[x][:], vtile,
                                            pt2[:, x * IC:(x + 1) * IC],
                                            start=(st == 0),
                                            stop=(st == ntot - 1))
                                    # denominator accumulation off the PE:
                                    # first NGP key tiles on gpsimd, rest DVE
                                    if st < NGP:
                                        nc.gpsimd.tensor_tensor(
                                            acc[1][:], acc[1][:], pt2[:], add)
                                    elif st == NGP:
                                        nc.vector.tensor_copy(acc[0][:], pt2[:])
                                    else:
                                        nc.vector.tensor_tensor(
                                            acc[0][:], acc[0][:], pt2[:], add)
                                # unnormalized o^T leaves PSUM immediately so
                                # the op banks recycle without waiting on any
                                # reduction chain
                                buf = a2a_in[g * B + b]
                                hwc = TPC // B
                                for x in range(2):
                                    osc = oscp.tile([128, IC], BF16, tag="osc")
                                    nc.scalar.activation(osc[:], op_ps[x][:],
                                                         Copy)
                                    for hh in range(2):
                                        r0 = (2 * ics[x] + hh) * (D + 2)
                                        nc.sync.dma_start(
                                            buf[r0:r0 + D, :],
                                            osc[:, hh * hwc:(hh + 1) * hwc])
                                nc.vector.tensor_tensor(acc[0][:], acc[0][:],
                                                        acc[1][:], add)
                                wo_prefetch(chunk_i)
                                chunk_i += 1
                                # denominator flush for the PREVIOUS chunk is
                                # emitted here so its PE matmuls never stall
                                # on this chunk's DVE merge
                                if pend is not None:
                                    den_flush(*pend)
                                pend = (b, g, ics, acc,
                                        icp == NIC // 2 - 1)
                    den_flush(*pend)

            # ---- S6: reconstruct denominators, broadcast their inverses
            # with K=1 PE matmuls, normalize each received buffer in one
            # wide DVE multiply, then o_proj in hc-pair phases ----
            with (
                tc.tile_pool(name="oTp", bufs=1) as oTp,
                tc.tile_pool(name="dhl", bufs=4) as dhlp,
                tc.tile_pool(name="dzp", bufs=8) as dzp,
                tc.tile_pool(name="outsb", bufs=4) as outsbp,
                tc.tile_pool(name="rbps", bufs=1, space="PSUM") as rbps,
                tc.tile_pool(name="opps2", bufs=2, space="PSUM") as opps2,
            ):
                hwc = TPC // B
                W = N_CORES * hwc
                otn = {}
                for b in range(B):
                    for g in range(G):
                        gb = g * B + b
                        src_t = a2a_out[gb].rearrange("(j p) n -> p j n",
                                                      p=D + 2)
                        otall = oTp.tile([128, N_CORES, hwc], BF16,
                                         name=f"otall{gb}")
                        nc.sync.dma_start(otall[:], src_t[0:D])
                        dhi = dhlp.tile([1, W], BF16, tag="dhi")
                        dlo = dhlp.tile([1, W], BF16, tag="dlo")
                        nc.sync.dma_start(
                            dhi[:].rearrange("p (j n) -> p j n", j=N_CORES),
                            src_t[D:D + 1])
                        nc.sync.dma_start(
                            dlo[:].rearrange("p (j n) -> p j n", j=N_CORES),
                            src_t[D + 1:D + 2])
                        rb_ps = rbps.tile([128, W], F32, tag="rb")
                        dns = []
                        for k in range(W // 512):
                            sl = slice(k * 512, (k + 1) * 512)
                            dn = dzp.tile([1, 512], F32, tag="dz")
                            nc.gpsimd.tensor_tensor(dn[:], dhi[:, sl],
                                                    dlo[:, sl],
                                                    mybir.AluOpType.add)
                            dns.append(dn)
                        for k in range(W // 512):
                            sl = slice(k * 512, (k + 1) * 512)
                            rc = dzp.tile([1, 512], F32, tag="rc")
                            nc.vector.reciprocal_approx_fast(rc[:], dns[k][:])
                            rcr = dzp.tile([1, 512], F32R, tag="rcr")
                            nc.vector.tensor_copy(rcr[:], rc[:])
                            nc.tensor.matmul(rb_ps[:, sl], onesr_t[:],
                                             rcr[:], start=True, stop=True)
                        o_n = oTp.tile([128, W], F32R, name=f"otn{gb}")
                        nc.vector.tensor_tensor(
                            o_n[:], otall[:].rearrange("p j n -> p (j n)"),
                            rb_ps[:], mybir.AluOpType.mult)
                        otn[(b, g)] = o_n
                for hcp in range(2):
                    for b in range(B):
                        for hc in (2 * hcp, 2 * hcp + 1):
                            for it in range(hwc // 128):
                                op_ps = opps2.tile([128, 512], F32, tag="oo")
                                for dt in range(NKT):
                                    j, g = dt // G, dt % G
                                    stat = otn[(b, g)][
                                        :, j * hwc + it * 128:
                                        j * hwc + (it + 1) * 128]
                                    nc.tensor.matmul(
                                        op_ps[:], stat, wo_t[(hc, dt)][:],
                                        start=(dt == 0), stop=(dt == NKT - 1))
                                ob = outsbp.tile([128, 512], F32, tag="ob")
                                nc.scalar.activation(ob[:], op_ps[:], Copy)
                                r0 = b * hwc + it * 128
                                nc.gpsimd.dma_start(
                                    out[r0:r0 + 128,
                                        hc * 512:(hc + 1) * 512], ob[:])
            wost_cm.__exit__(None, None, None)

    nc.compile()
    return nc


def kernel(positions, hidden_states, idx_salient, k_cache_prev, v_cache_prev,
           Wq, bq, Wkv, bkv, Wo):
    pos = np.asarray(positions).astype(np.int64)
    hs = np.asarray(hidden_states, dtype=np.float32)
    idx = np.asarray(idx_salient).astype(np.int64)
    kc = np.asarray(k_cache_prev, dtype=np.float32)
    vc = np.asarray(v_cache_prev, dtype=np.float32)
    Wq = np.asarray(Wq, dtype=np.float32)
    bq = np.asarray(bq, dtype=np.float32)
    Wkv = np.asarray(Wkv, dtype=np.float32)
    bkv = np.asarray(bkv, dtype=np.float32)
    Wo = np.asarray(Wo, dtype=np.float32)

    FP8NP = ml_dtypes.float8_e4m3

    # dedup (keep last occurrence, matching scatter-set semantics) and
    # partition the salient rows by batch, 128-aligned per batch
    _, last_pos = np.unique(idx[::-1], return_index=True)
    keep_mask = np.zeros(len(idx), bool)
    keep_mask[len(idx) - 1 - last_pos] = True
    idx_u = idx[keep_mask]
    n0 = int(np.searchsorted(idx_u, L))
    n1 = len(idx_u) - n0
    t0 = max(1, (n0 + 127) // 128)
    t1 = max(1, (n1 + 127) // 128)
    SJT = t0 + t1
    SSL = SJT * 128
    SP = ((SSL + 511) // 512) * 512
    NTB = NST + max(t0, t1)

    key = (t0, t1)
    if key not in _cache:
        _cache[key] = _build(t0, t1)
    nc = _cache[key]

    # slot s -> source token (or -1 for pad)
    slot_src = np.full(SSL, -1, np.int64)
    slot_src[0:n0] = idx_u[0:n0]
    slot_src[t0 * 128:t0 * 128 + n1] = idx_u[n0:]
    valid = slot_src >= 0
    src = np.where(valid, slot_src, 0)

    hT = np.ascontiguousarray(hs.T * HSC).astype(FP8NP)
    hsal = np.where(valid[:, None], hs[src], 0.0)           # [SSL, HIDDEN]
    hsalT = np.zeros((HIDDEN + 1, SP), np.float32)
    hsalT[:HIDDEN, :SSL] = hsal.T * HSC
    hsalT[HIDDEN, :] = HSC
    hsalT = hsalT.astype(FP8NP)

    inv_freq = 1.0 / (ROPE_BASE ** (np.arange(HALF, dtype=np.float64) / HALF))
    ang_q = np.outer(inv_freq, pos.astype(np.float64))
    csq1_h = np.concatenate([np.cos(ang_q), np.cos(ang_q)]).astype(ml_dtypes.bfloat16)
    csq2_h = np.concatenate([-np.sin(ang_q), np.sin(ang_q)]).astype(ml_dtypes.bfloat16)
    pos_sl = np.where(valid, pos[src], 0)
    ang_s = np.outer(inv_freq, pos_sl.astype(np.float64))
    css1_h = np.zeros((D, SP), np.float32)
    css2_h = np.zeros((D, SP), np.float32)
    css1_h[:, :SSL] = np.concatenate([np.cos(ang_s), np.cos(ang_s)])
    css2_h[:, :SSL] = np.concatenate([-np.sin(ang_s), np.sin(ang_s)])
    css1_h = css1_h.astype(ml_dtypes.bfloat16)
    css2_h = css2_h.astype(ml_dtypes.bfloat16)

    swm_h = np.zeros((D, D), np.float32)
    swm_h[np.arange(D), (np.arange(D) + HALF) % D] = 1.0
    kv_size = HKV * D

    # additive exp bias per attended key tile: prev tiles get 0 (kept) /
    # NEG (stale salient row); salient tiles get 0 (valid) / NEG (pad)
    keep = np.ones(T, np.float32)
    keep[idx_u] = 0.0
    abias_h = np.zeros((B, 128, NTB), np.float32)
    for b in range(B):
        kb = keep[b * L:(b + 1) * L].reshape(NST, 128).T     # [128, 16]
        abias_h[b, :, :NST] = np.where(kb > 0, 0.0, NEG)
        tb = t0 if b == 0 else t1
        off = 0 if b == 0 else t0
        vb = valid[off * 128:(off + tb) * 128].reshape(tb, 128).T
        abias_h[b, :, NST:NST + tb] = np.where(vb, 0.0, NEG)

    def pack_w(wcols):
        # [HIDDEN, M] -> [128, NKP, 2, M] DoubleRow stationary layout
        m = wcols.shape[1]
        return np.ascontiguousarray(
            (wcols * WSC).reshape(NKP, 2, 128, m).transpose(2, 0, 1, 3)
        ).astype(FP8NP)

    in_maps = []
    for c in range(N_CORES):
        kcc = kc[:, c, :]
        kpT_h = np.stack([np.ascontiguousarray(kcc[b * L:(b + 1) * L].T)
                          for b in range(B)]).astype(ml_dtypes.bfloat16)
        vcc = vc[:, c, :]
        vpa_h = np.stack([vcc[b * L:(b + 1) * L] for b in range(B)]
                         ).astype(ml_dtypes.bfloat16)
        in_maps.append({
            "hT": hT,
            "hsalT": hsalT,
            "wq": pack_w(Wq[:, c * DOUT:(c + 1) * DOUT]),
            "bq": np.ascontiguousarray(
                bq[c * DOUT:(c + 1) * DOUT].reshape(G, 128, 1)),
            "wk": pack_w(Wkv[:, c * D:(c + 1) * D]),
            "bk": np.ascontiguousarray(bkv[c * D:(c + 1) * D].reshape(128, 1)),
            "wv": pack_w(Wkv[:, kv_size + c * D:kv_size + (c + 1) * D]),
            "wvl": (bkv[kv_size + c * D:kv_size + (c + 1) * D]
                    .reshape(1, D) * WSC).astype(FP8NP),
            "wo": Wo,
            "kpT": kpT_h,
            "vpa": vpa_h,
            "onec": np.ones((128, 1), np.float32).astype(ml_dtypes.bfloat16),
            "onesr": np.ones((1, 128), np.float32),
            "csq1": csq1_h,
            "csq2": csq2_h,
            "css1": css1_h,
            "css2": css2_h,
            "swm": swm_h.astype(ml_dtypes.bfloat16),
            "idm": np.eye(D, dtype=np.float32).astype(ml_dtypes.bfloat16),
            "abias": abias_h,
        })

    res = bass_utils.run_bass_kernel_spmd(nc, in_maps,
                                          core_ids=list(range(N_CORES)))
    # core c's "out" rows: [0:256] = batch-0 tokens c*256.., [256:512] =
    # batch-1 tokens 2048 + c*256..
    half = TPC // B
    full = np.empty((T, HIDDEN), np.float32)
    for c in range(N_CORES):
        o = res.results[c]["out"]
        full[c * half:(c + 1) * half] = o[0:half]
        full[L + c * half:L + (c + 1) * half] = o[half:TPC]
    return full
